# revision 14
# baseline (speedup 1.0000x reference)
"""Encoder-decoder LSTM seq2seq loss kernel for 8 TRN2 NeuronCores.

Strategy:
  - LSTM recurrences (encoder 48 steps, decoder 47 steps) are replicated on
    every core in gate-major layout: gates^T [2048, 64] computed as 16
    [128,64] PSUM chunks, state kept transposed (hT [128, 4*64]) so no
    per-step transposes are needed.
  - Input-side gate contributions (x @ W_ih^T + b) are batched in 8-step
    windows as full-utilization [128,128]x[128,512] matmuls interleaved
    between recurrence steps; the per-step x-injection into the gates
    PSUM is an identity matmul on the PE (exact for 1.0*bf16).
  - The per-step chain: gate groups issue G, F, I, O; the decoder splits
    the i/f sigmoid so t1 = sig_f*c (Pool) and t2 = sig_i*tanh_g (DVE)
    start while the O matmuls stream; the h = sig_o*tanh(c) tail is
    split across DVE (k01) and Pool (k23) so the next step's k01
    matmuls start early. The encoder (more DVE load: masks) keeps a
    merged i|f sigmoid and whole-width predicated restores.
  - All heavy prefetches (decoder weights, fp8 W_out shard, fp8
    gathered-target W_out, scaled output bias) are issued on the ACT
    engine's separate DMA queue two steps into the recurrence, so they
    neither delay the recurrence-critical startup DMAs nor block the
    per-step mask/window DMAs on the sync queue.
  - Target logits are computed incrementally during the decoder (one
    512-column piece per 8 steps: DVE/Pool elementwise muls + ones
    contraction on the PE) so no separate phase remains.
  - The 32k-vocab logits GEMM runs as fp8 DoubleRow matmuls (2x fewer
    PE cycles): ht is converted per sb-chunk to scaled fp8 (x32), W_out
    shard is host-scaled fp8 (x256); the output bias (x8192) is
    injected by K=1 ones-matmuls as the PSUM accumulation start, and
    the 1/8192 rescale is folded into the ACT Exp's scale, whose
    free-axis accumulator produces the softmax denominator directly.
    Per [128,2048] PSUM half: 4 bias MMs + 8 DoubleRow MMs -> Exp.
  - Host combines per-core partial sum-exp + target logits into the
    scalar loss (tiny: 8 x [128,24] + [1,3008]).
"""

import sys

sys.path.insert(0, "/opt/trn_rl_repo")

import numpy as np
import ml_dtypes

BF16 = ml_dtypes.bfloat16
FP8 = ml_dtypes.float8_e4m3

# Model dims (hardcoded per contract)
SRC, TGT, B, H, V = 48, 48, 64, 512, 32000
DEC = TGT - 1                  # 47 decoder steps
SB = DEC * B                   # 3008 (step*batch)
SBC = 24                       # ceil(3008/128) sb-chunks
SBP = SBC * 128                # 3072 padded
NCORES = 8
VSH = V // NCORES              # 4000 vocab rows per core
VSP = 4096                     # padded shard
WIN = 8                        # bulk x-part window (steps)
NG = 16                        # gate chunks (2048/128)
KC = 4                         # hidden chunks (512/128)

H_SC = 32.0                    # fp8 scale for ht
XE_SC = 256.0                  # fp8 scale for x embeddings
WI_SC = 256.0                  # fp8 scale for W_ih
X_SC = XE_SC * WI_SC           # x-part gate rescale (65536)
W_SC = 256.0                   # fp8 scale for W_out / W_out[tgt]
L_SC = H_SC * W_SC             # logits scale (8192)

# gate-chunk indices in the permuted [i f o g] weight layout
I_CH = list(range(0, 4))
F_CH = list(range(4, 8))
O_CH = list(range(8, 12))
G_CH = list(range(12, 16))

_COMPILED = None


def _build():
    import concourse.bass as bass
    import concourse.bacc as bacc
    import concourse.tile as tile
    from concourse import mybir

    f32 = mybir.dt.float32
    bf16 = mybir.dt.bfloat16
    fp8 = mybir.dt.float8e4
    AF = mybir.ActivationFunctionType
    DR = mybir.MatmulPerfMode.DoubleRow

    nc = bacc.Bacc("TRN2", target_bir_lowering=False, debug=False,
                   num_devices=NCORES)

    def din(name, shape, dt=bf16):
        return nc.dram_tensor(name, shape, dt, kind="ExternalInput").ap()

    xt_enc = din("xt_enc", [H, SRC * B])
    xt_dec = din("xt_dec", [H, SB])
    wi_e = din("wi_e", [KC, 128, 4 * H])
    wh_e = din("wh_e", [KC, 128, 4 * H])
    wi_d = din("wi_d", [KC, 128, 4 * H])
    wh_d = din("wh_d", [KC, 128, 4 * H])
    bias_e = din("bias_e", [128, NG], f32)
    bias_d = din("bias_d", [128, NG], f32)
    mask_in = din("mask", [SRC, 128, KC * B], mybir.dt.uint8)
    ident_in = din("ident", [128, 128])
    wot_in = din("wot", [KC, 128, VSP], fp8)
    bout_in = din("bout_sc", [128, VSP])
    wtgt_in = din("wtgt", [KC, 128, SB], fp8)

    out_s = nc.dram_tensor("out_s", [128, SBC], f32, kind="ExternalOutput").ap()
    out_l = nc.dram_tensor("out_l", [1, SB], f32, kind="ExternalOutput").ap()

    with tile.TileContext(nc) as tc:
        from contextlib import ExitStack
        with ExitStack() as ctx:
            # ---- pools ----
            pconst = ctx.enter_context(tc.tile_pool(name="const", bufs=1))
            pht = ctx.enter_context(tc.tile_pool(name="ht", bufs=1))
            pgx = ctx.enter_context(tc.tile_pool(name="gx", bufs=2))
            pw = ctx.enter_context(tc.tile_pool(name="w", bufs=1))
            pwt = ctx.enter_context(tc.tile_pool(name="wt", bufs=1))
            pwo = ctx.enter_context(tc.tile_pool(name="wo", bufs=1))
            pxt = ctx.enter_context(tc.tile_pool(name="xtw", bufs=2))
            pstate = ctx.enter_context(tc.tile_pool(name="state", bufs=3))
            pact = ctx.enter_context(tc.tile_pool(name="act", bufs=2))
            pmask = ctx.enter_context(tc.tile_pool(name="mask", bufs=2))
            plog = ctx.enter_context(tc.tile_pool(name="log", bufs=2))
            pfp = ctx.enter_context(tc.tile_pool(name="fp", bufs=4))

            # ---- constants ----
            def dve_const(src_ap, shape, dt, tag):
                dma_t = pconst.tile(shape, dt, tag=f"{tag}_dma")
                nc.sync.dma_start(dma_t[:], src_ap)
                t = pconst.tile(shape, dt, tag=tag)
                nc.vector.tensor_copy(t[:], dma_t[:])
                return t

            bias_e_t = dve_const(bias_e[:], [128, NG], f32, "be")
            bias_d_t = dve_const(bias_d[:], [128, NG], f32, "bd")
            ones_t = pconst.tile([128, 1], f32)
            nc.vector.memset(ones_t[:], 1.0)
            ones_row = pconst.tile([1, 128], bf16)
            nc.vector.memset(ones_row[:], 1.0)
            ident = pconst.tile([128, 128], bf16)
            nc.sync.dma_start(ident[:], ident_in[:])

            # deferred-prefetch tiles (DMAs issued on the ACT engine's DMA
            # queue at step 2 so they don't block sync-queue DMAs)
            wtg_tiles = [pwt.tile([128, 2 * SB], fp8, tag=f"wtg{h}",
                                  name=f"wtg{h}") for h in range(2)]
            wtg_s = lambda k: wtg_tiles[k // 2][:, (k % 2) * SB:
                                                (k % 2 + 1) * SB]
            wof_tiles = [pwo.tile([128, 2 * VSP], fp8, tag=f"wof{h}",
                                  name=f"wof{h}") for h in range(2)]
            bout = pconst.tile([128, VSP], bf16)

            def load_w(dram, pool, tag, eng, width=4 * H, eng2=None):
                ts = []
                dw = dram.shape[2]
                for k in range(KC):
                    t = pool.tile([128, width], bf16, tag=f"{tag}{k}")
                    e = eng if (eng2 is None or k < 2) else eng2
                    e.dma_start(t[:, :dw], dram[k])
                    ts.append(t)
                return ts

            wd = {}

            def prefetch_all():
                wd["i"] = load_w(wi_d, pw, "wid", nc.scalar)
                wd["h"] = load_w(wh_d, pw, "whd", nc.scalar)
                for half in range(2):
                    for j in range(2):
                        nc.scalar.dma_start(
                            wtg_tiles[half][:, j * SB:(j + 1) * SB],
                            wtgt_in[half * 2 + j])
                        nc.scalar.dma_start(
                            wof_tiles[half][:, j * VSP:(j + 1) * VSP],
                            wot_in[half * 2 + j])
                nc.scalar.dma_start(bout[:], bout_in[:])

            # HT: decoder hidden states, transposed, col = k*SBP + t*64 + b
            ht = pht.tile([128, KC * SBP], bf16)
            nc.vector.memset(ht[:], 0.0)

            we_i = load_w(wi_e, pw, "wie", nc.sync, eng2=nc.scalar)
            we_h = None   # loaded after the prologue window's xt DMA

            l_sb = pconst.tile([1, SB], f32)

            # ============ unified 95-step recurrence ============
            with (
                tc.tile_pool(name="psA", bufs=3, space=bass.MemorySpace.PSUM)
                    as psA,
                tc.tile_pool(name="psB", bufs=2, space=bass.MemorySpace.PSUM)
                    as psB,
                tc.tile_pool(name="psC", bufs=2, space=bass.MemorySpace.PSUM)
                    as psC,
                tc.tile_pool(name="psT", bufs=1, space=bass.MemorySpace.PSUM)
                    as psT,
            ):
                def bulk_pieces(xt_src, wkey, bias_t, t0, nsteps):
                    """Yield closures: piece 0 = DMA + gx alloc, one piece
                    per gate chunk (4 MMs + bias fold to gx), then a
                    sentinel returning the gx tile."""
                    wi_t = we_i if wkey == "enc" else wd["i"]
                    w = nsteps * B
                    state = {}

                    def p_dma():
                        state["gx"] = pgx.tile([128, NG * WIN * B], bf16,
                                               tag="gx", name="gxw")
                        xtw = []
                        for k in range(KC):
                            t = pxt.tile([128, WIN * B], bf16, tag=f"xt{k}")
                            nc.sync.dma_start(
                                t[:, :w], xt_src[k * 128:(k + 1) * 128,
                                                 t0 * B:t0 * B + w])
                            xtw.append(t)
                        state["xtw"] = xtw
                    yield p_dma

                    def mk_chunk(g):
                        def p_mm():
                            pb = psA.tile([128, 512], f32, tag="psA")
                            for k in range(KC):
                                nc.tensor.matmul(
                                    pb[:, :w],
                                    wi_t[k][:, g * 128:(g + 1) * 128],
                                    state["xtw"][k][:, :w],
                                    start=(k == 0), stop=(k == KC - 1))
                            gxs = state["gx"][:, g * WIN * B:g * WIN * B + w]

                            def p_bias():
                                # deferred to the next step's top so the
                                # 700ns bias op never sits in the engine
                                # FIFO ahead of chain ops it would block
                                if wkey == "dec" or g % 2 == 0:
                                    nc.vector.tensor_scalar_add(
                                        gxs, pb[:, :w], bias_t[:, g:g + 1])
                                else:
                                    nc.scalar.activation(
                                        gxs, pb[:, :w], AF.Identity,
                                        bias=bias_t[:, g:g + 1])
                            return p_bias
                        return p_mm
                    for g in range(NG):
                        yield mk_chunk(g)
                    yield lambda: state["gx"]

                def tgt_piece(nt):
                    """One 512-col piece of the gathered-target logits:
                    l[s] = sum_h ht[h,s]*wtgt[h,s] (DVE/Pool muls + ones
                    contraction), scaled back from the fp8 W scale."""
                    wdt = min(512, SB - nt * 512)
                    pt = psT.tile([128, 512], f32, tag="psT")
                    for k in range(KC):
                        prod = plog.tile([128, 512], f32, tag=f"prod{k % 2}",
                                         name="prod")
                        eng = nc.vector if k % 2 == 0 else nc.gpsimd
                        eng.tensor_mul(
                            prod[:, :wdt],
                            ht[:, k * SBP + nt * 512:k * SBP + nt * 512 + wdt],
                            wtg_s(k)[:, nt * 512:nt * 512 + wdt])
                        nc.tensor.matmul(pt[0:1, :wdt], ones_t[:],
                                         prod[:, :wdt],
                                         start=(k == 0), stop=(k == KC - 1))
                    nc.scalar.activation(l_sb[:, nt * 512:nt * 512 + wdt],
                                         pt[0:1, :wdt], AF.Copy,
                                         scale=1.0 / W_SC)

                def lstm_step(gx, lt, h_rhs, c_prev, wh_t, out01, out23,
                              split_sig):
                    pA = psA.tile([128, 512], f32, tag="psA")  # i|f
                    pB = psB.tile([128, 256], f32, tag="psB")  # g
                    pC = psC.tile([128, 256], f32, tag="psC")  # o

                    gx_r = gx[:].rearrange("p (g s) -> p g s", g=NG)
                    nc.tensor.matmul(
                        pB[:].rearrange("p (g s) -> p g s", g=4),
                        ident[:], gx_r[:, 12:16, lt * B:(lt + 1) * B],
                        start=True, stop=False)
                    nc.tensor.matmul(
                        pA[:].rearrange("p (g s) -> p g s", g=8),
                        ident[:], gx_r[:, 0:8, lt * B:(lt + 1) * B],
                        start=True, stop=False)
                    nc.tensor.matmul(
                        pC[:].rearrange("p (g s) -> p g s", g=4),
                        ident[:], gx_r[:, 8:12, lt * B:(lt + 1) * B],
                        start=True, stop=False)

                    def mm(c, k, stop):
                        if c in G_CH:
                            dst = pB[:, (c - 12) * B:(c - 11) * B]
                        elif c in O_CH:
                            dst = pC[:, (c - 8) * B:(c - 7) * B]
                        else:
                            dst = pA[:, c * B:(c + 1) * B]
                        nc.tensor.matmul(dst,
                                         wh_t[k][:, c * 128:(c + 1) * 128],
                                         h_rhs(k), start=False, stop=stop)

                    # G group first (k01 before k23: h halves arrive
                    # staggered from the previous step's split tail)
                    for k in (0, 1):
                        for c in G_CH:
                            mm(c, k, False)
                    for k in (2, 3):
                        for c in G_CH:
                            mm(c, k, (k == 3 and c == 15))
                    tng = pact.tile([128, 256], bf16, tag="tng")
                    nc.scalar.activation(tng[:], pB[:], AF.Tanh)
                    # F group
                    for c in F_CH:
                        for k in range(KC):
                            mm(c, k, False)
                    sig = pact.tile([128, 512], bf16, tag="sig")
                    if split_sig:
                        nc.scalar.activation(sig[:, 256:512], pA[:, 256:512],
                                             AF.Sigmoid)
                        t1 = pact.tile([128, 256], bf16, tag="t1")
                        nc.gpsimd.tensor_mul(t1[:], sig[:, 256:512],
                                             c_prev[:])
                    # I group
                    for c in I_CH:
                        for k in range(KC):
                            mm(c, k, (c == 3 and k == 3))
                    if split_sig:
                        nc.scalar.activation(sig[:, 0:256], pA[:, 0:256],
                                             AF.Sigmoid)
                    else:
                        nc.scalar.activation(sig[:], pA[:], AF.Sigmoid)
                        t1 = pact.tile([128, 256], bf16, tag="t1")
                        nc.gpsimd.tensor_mul(t1[:], sig[:, 256:512],
                                             c_prev[:])
                    t2 = pact.tile([128, 256], bf16, tag="t2")
                    nc.vector.tensor_mul(t2[:], sig[:, 0:256], tng[:])
                    # O group
                    for c in O_CH:
                        for k in range(KC):
                            mm(c, k, (c == 11 and k == 3))
                    c_new = pstate.tile([128, 256], bf16, tag="c")
                    nc.vector.tensor_add(c_new[:], t1[:], t2[:])
                    sgo = pact.tile([128, 256], bf16, tag="sgo")
                    nc.scalar.activation(sgo[:], pC[:], AF.Sigmoid)
                    tnc = pact.tile([128, 256], bf16, tag="tnc")
                    nc.scalar.activation(tnc[:], c_new[:], AF.Tanh)
                    # h = sgo*tnc: halves on DVE (k01) and Pool (k23)
                    nc.vector.tensor_mul(out01, sgo[:, 0:128], tnc[:, 0:128])
                    nc.gpsimd.tensor_mul(out23, sgo[:, 128:256],
                                         tnc[:, 128:256])
                    return c_new

                h_prev = pstate.tile([128, KC * B], bf16, tag="h")
                nc.vector.memset(h_prev[:], 0.0)
                c_prev = pstate.tile([128, 256], bf16, tag="c")
                nc.vector.memset(c_prev[:], 0.0)

                win_list = (
                    [(xt_enc, "enc", bias_e_t, t0, min(WIN, SRC - t0))
                     for t0 in range(0, SRC, WIN)] +
                    [(xt_dec, "dec", bias_d_t, t0, min(WIN, DEC - t0))
                     for t0 in range(0, DEC, WIN)])

                pending = []

                def run_piece(p):
                    r = p()
                    if callable(r):
                        pending.append(r)
                        return None
                    return r

                def flush_pending():
                    for b in pending:
                        b()
                    pending.clear()

                gx = None
                pro_gen = bulk_pieces(*win_list[0])   # prologue window
                next(pro_gen)()                       # xt DMA first
                we_h = load_w(wh_e, pw, "whe", nc.sync, eng2=nc.scalar)
                for p in pro_gen:
                    r = run_piece(p)
                    gx = r if r is not None else gx
                flush_pending()
                next_idx = 1
                next_gen = bulk_pieces(*win_list[next_idx])
                gx_next = None

                ht_r = ht[:].rearrange("p (k s) -> p k s", k=KC)
                step_no = 0
                for phase, nsteps in (("enc", SRC), ("dec", DEC)):
                    wh_t = we_h if phase == "enc" else wd["h"]
                    for t in range(nsteps):
                        if t % WIN == 0 and step_no > 0:
                            while next_gen is not None:
                                try:
                                    p = next(next_gen)
                                except StopIteration:
                                    next_gen = None
                                    break
                                r = run_piece(p)
                                gx_next = r if r is not None else gx_next
                            flush_pending()
                            gx, gx_next = gx_next, None
                            next_idx += 1
                            if next_idx < len(win_list):
                                next_gen = bulk_pieces(*win_list[next_idx])
                        else:
                            flush_pending()
                        if phase == "dec" and t % WIN == 0 and t > 0:
                            tgt_piece(t // WIN - 1)
                        if phase == "enc" or t == 0:
                            hp = h_prev
                            rhs = (lambda k, hp=hp:
                                   hp[:, k * B:(k + 1) * B])
                        else:
                            rhs = (lambda k, tp=t - 1:
                                   ht[:, k * SBP + tp * B:
                                      k * SBP + (tp + 1) * B])
                        if phase == "enc":
                            h_new = pstate.tile([128, KC * B], bf16, tag="h")
                            out01 = h_new[:, 0:128]
                            out23 = h_new[:, 128:256]
                        else:
                            out01 = ht_r[:, 0:2, t * B:(t + 1) * B]
                            out23 = ht_r[:, 2:4, t * B:(t + 1) * B]
                        c_new = lstm_step(gx, t % WIN, rhs, c_prev, wh_t,
                                          out01, out23,
                                          split_sig=(phase == "dec"))
                        if phase == "enc":
                            mk = pmask.tile([128, KC * B], mybir.dt.uint8,
                                            tag="mk")
                            nc.sync.dma_start(mk[:], mask_in[t])
                            nc.vector.copy_predicated(h_new[:], mk[:],
                                                      h_prev[:])
                            nc.vector.copy_predicated(c_new[:], mk[:],
                                                      c_prev[:])
                            h_prev = h_new
                        c_prev = c_new
                        step_no += 1
                        if step_no == 2:
                            prefetch_all()
                        if next_gen is not None:
                            for _ in range(2):
                                try:
                                    p = next(next_gen)
                                except StopIteration:
                                    next_gen = None
                                    break
                                r = run_piece(p)
                                gx_next = r if r is not None else gx_next
                flush_pending()
                tgt_piece(5)
                nc.sync.dma_start(out_l[:], l_sb[:])

            # ============ fp8 DoubleRow vocab logits + sum-exp ============
            with tc.tile_pool(name="psL", bufs=2,
                              space=bass.MemorySpace.PSUM) as psL:
                s_all = pconst.tile([128, SBC], f32)

                for sb in range(SBC):
                    st = pfp.tile([128, 512], fp8, tag="st")
                    nc.vector.tensor_scalar_mul(
                        st[:].rearrange("p (k s) -> p k s", k=KC),
                        ht_r[:, :, sb * 128:(sb + 1) * 128], H_SC)
                    sh = []
                    for half in range(2):
                        pl = psL.tile([128, 2048], f32, tag="psL")
                        pe_bias = (half == 0)
                        if pe_bias:
                            for v in range(4):
                                col = half * 2048 + v * 512
                                nc.tensor.matmul(
                                    pl[:, v * 512:(v + 1) * 512],
                                    ones_row[:], bout[0:1, col:col + 512],
                                    start=True, stop=False)
                        for pair in range(2):
                            lhs = st[:, pair * 256:(pair + 1) * 256]\
                                .rearrange("p (two m) -> p two m", two=2)
                            for v in range(4):
                                col = half * 2048 + v * 512
                                rhs = wof_tiles[pair][:]\
                                    .rearrange("p (two v) -> p two v", two=2)\
                                    [:, :, col:col + 512]
                                nc.tensor.matmul(
                                    pl[:, v * 512:(v + 1) * 512], lhs, rhs,
                                    start=(not pe_bias and pair == 0),
                                    stop=(pair == 1), perf_mode=DR)
                        if not pe_bias:
                            nc.vector.tensor_add(
                                pl[:], pl[:],
                                bout[:, half * 2048:half * 2048 + 2048])
                        sh_t = plog.tile([128, 1], f32, tag=f"sh{half}",
                                         name="sh_t")
                        nc.scalar.activation(pl[:], pl[:], AF.Exp,
                                             scale=1.0 / L_SC,
                                             accum_out=sh_t[:])
                        sh.append(sh_t)
                    nc.gpsimd.tensor_add(s_all[:, sb:sb + 1],
                                         sh[0][:], sh[1][:])
                nc.sync.dma_start(out_s[:], s_all[:])

    nc.compile()
    return nc


def _prep(inputs):
    """Host-side data prep. Returns per-core in_maps + host combine data."""
    il = np.asarray(inputs["input_lines"])
    tl = np.asarray(inputs["target_lines"])
    f = lambda k: np.asarray(inputs[k], np.float32)
    emb_in, emb_tgt = f("emb_in").copy(), f("emb_tgt").copy()
    emb_in[0] = 0.0
    emb_tgt[0] = 0.0
    W_out, b_out = f("W_out"), f("b_out")

    perm = np.concatenate([np.arange(0, 512), np.arange(512, 1024),
                           np.arange(1536, 2048), np.arange(1024, 1536)])

    def wt(w):  # [2048,512] -> [4,128,2048] bf16 (transposed, gate-permuted)
        return np.ascontiguousarray(
            w[perm].T.reshape(KC, 128, 4 * H)).astype(BF16)

    def wt_pair(w):  # -> [2,128,2*2048] fp8 (k-pairs side by side, scaled)
        r = np.clip(w[perm].T * WI_SC, -240, 240).reshape(KC, 128, 4 * H)
        pairs = np.stack([
            np.concatenate([r[0], r[1]], axis=1),
            np.concatenate([r[2], r[3]], axis=1)])
        return np.ascontiguousarray(pairs).astype(FP8)

    def bias(bi, bh):  # -> [128, 16] f32
        return np.ascontiguousarray(
            (bi + bh)[perm].reshape(NG, 128).T).astype(np.float32)

    x_enc = emb_in[il.reshape(-1)]                       # [3072, 512]
    xt_enc = np.ascontiguousarray(x_enc.T).astype(BF16)  # [512, 3072]
    tgt_in = tl[:DEC].reshape(-1)
    x_dec = emb_tgt[tgt_in]
    xt_dec = np.ascontiguousarray(x_dec.T).astype(BF16)  # [512, 3008]

    m = (il == 0).astype(np.uint8)                       # [48, 64]
    mask = np.ascontiguousarray(np.broadcast_to(
        m[:, None, None, :], (SRC, 128, KC, B)).reshape(
            SRC, 128, KC * B)).astype(np.uint8)

    tgt_next = tl[1:TGT].reshape(-1)                     # [3008]
    wtgt = np.ascontiguousarray(np.clip(
        W_out[tgt_next].T * W_SC, -240, 240).reshape(
            KC, 128, SB)).astype(FP8)
    b_tgt = b_out[tgt_next].astype(np.float64)

    common = dict(
        xt_enc=xt_enc, xt_dec=xt_dec,
        wi_e=wt(f("W_ih_e")), wh_e=wt(f("W_hh_e")),
        wi_d=wt(f("W_ih_d")), wh_d=wt(f("W_hh_d")),
        bias_e=bias(f("b_ih_e"), f("b_hh_e")),
        bias_d=bias(f("b_ih_d"), f("b_hh_d")),
        mask=mask, wtgt=wtgt,
        ident=np.eye(128, dtype=BF16),
    )
    in_maps = []
    for c in range(NCORES):
        ws = np.zeros((VSP, H), np.float32)
        ws[:VSH] = W_out[c * VSH:(c + 1) * VSH]
        bs = np.full(VSP, -88.0 * L_SC, np.float32)
        bs[:VSH] = b_out[c * VSH:(c + 1) * VSH] * L_SC
        in_maps.append(dict(
            common,
            wot=np.ascontiguousarray(np.clip(
                ws.T * W_SC, -240, 240).reshape(KC, 128, VSP)).astype(FP8),
            bout_sc=np.ascontiguousarray(
                np.broadcast_to(bs, (128, VSP))).astype(BF16),
        ))
    return in_maps, b_tgt


def _combine(results, b_tgt):
    s = np.zeros(SBP, np.float64)
    for r in results:
        s += np.asarray(r["out_s"], np.float64).T.reshape(-1)
    s = s[:SB]
    lse = np.log(s)
    l_tgt = np.asarray(results[0]["out_l"], np.float64).reshape(-1) + b_tgt
    return np.float32((lse - l_tgt).sum() / B)


def kernel(**inputs):
    global _COMPILED
    from concourse.bass_utils import run_bass_kernel_spmd
    in_maps, b_tgt = _prep(inputs)
    if _COMPILED is None:
        _COMPILED = _build()
    res = run_bass_kernel_spmd(_COMPILED, in_maps, list(range(NCORES)))
    return _combine(res.results, b_tgt)


if __name__ == "__main__":
    import reference
    inp = reference.setup_inputs()
    expected = np.asarray(reference.reference(**inp))
    actual = kernel(**{k: np.asarray(v) for k, v in inp.items()})
    err = abs(actual - expected) / max(abs(expected), 1e-9)
    print(f"expected={expected} actual={actual} rel_err={err:.3e}")


# revision 15
# speedup vs baseline: 1.0005x; 1.0005x over previous
"""Encoder-decoder LSTM seq2seq loss kernel for 8 TRN2 NeuronCores.

Strategy:
  - LSTM recurrences (encoder 48 steps, decoder 47 steps) are replicated on
    every core in gate-major layout: gates^T [2048, 64] computed as 16
    [128,64] PSUM chunks, state kept transposed (hT [128, 4*64]) so no
    per-step transposes are needed.
  - Input-side gate contributions (x @ W_ih^T + b) are batched in 8-step
    windows as full-utilization [128,128]x[128,512] matmuls interleaved
    between recurrence steps; the per-step x-injection into the gates
    PSUM is an identity matmul on the PE (exact for 1.0*bf16).
  - The per-step chain: gate groups issue G, F, I, O; the decoder splits
    the i/f sigmoid so t1 = sig_f*c (Pool) and t2 = sig_i*tanh_g (DVE)
    start while the O matmuls stream; the h = sig_o*tanh(c) tail is
    split across DVE (k01) and Pool (k23) so the next step's k01
    matmuls start early. The encoder (more DVE load: masks) keeps a
    merged i|f sigmoid and whole-width predicated restores.
  - All heavy prefetches (decoder weights, fp8 W_out shard, fp8
    gathered-target W_out, scaled output bias) are issued on the ACT
    engine's separate DMA queue two steps into the recurrence, so they
    neither delay the recurrence-critical startup DMAs nor block the
    per-step mask/window DMAs on the sync queue.
  - Target logits are computed incrementally during the decoder (one
    512-column piece per 8 steps: DVE/Pool elementwise muls + ones
    contraction on the PE) so no separate phase remains.
  - The 32k-vocab logits GEMM runs as fp8 DoubleRow matmuls (2x fewer
    PE cycles): ht is converted per sb-chunk to scaled fp8 (x32), W_out
    shard is host-scaled fp8 (x256); the output bias (x8192) is
    injected by K=1 ones-matmuls as the PSUM accumulation start, and
    the 1/8192 rescale is folded into the ACT Exp's scale, whose
    free-axis accumulator produces the softmax denominator directly.
    Per [128,2048] PSUM half: 4 bias MMs + 8 DoubleRow MMs -> Exp.
  - Host combines per-core partial sum-exp + target logits into the
    scalar loss (tiny: 8 x [128,24] + [1,3008]).
"""

import sys

sys.path.insert(0, "/opt/trn_rl_repo")

import numpy as np
import ml_dtypes

BF16 = ml_dtypes.bfloat16
FP8 = ml_dtypes.float8_e4m3

# Model dims (hardcoded per contract)
SRC, TGT, B, H, V = 48, 48, 64, 512, 32000
DEC = TGT - 1                  # 47 decoder steps
SB = DEC * B                   # 3008 (step*batch)
SBC = 24                       # ceil(3008/128) sb-chunks
SBP = SBC * 128                # 3072 padded
NCORES = 8
VSH = V // NCORES              # 4000 vocab rows per core
VSP = 4096                     # padded shard
WIN = 8                        # bulk x-part window (steps)
NG = 16                        # gate chunks (2048/128)
KC = 4                         # hidden chunks (512/128)

H_SC = 32.0                    # fp8 scale for ht
XE_SC = 256.0                  # fp8 scale for x embeddings
WI_SC = 256.0                  # fp8 scale for W_ih
X_SC = XE_SC * WI_SC           # x-part gate rescale (65536)
W_SC = 256.0                   # fp8 scale for W_out / W_out[tgt]
L_SC = H_SC * W_SC             # logits scale (8192)

# gate-chunk indices in the permuted [i f o g] weight layout
I_CH = list(range(0, 4))
F_CH = list(range(4, 8))
O_CH = list(range(8, 12))
G_CH = list(range(12, 16))

_COMPILED = None


def _build():
    import concourse.bass as bass
    import concourse.bacc as bacc
    import concourse.tile as tile
    from concourse import mybir

    f32 = mybir.dt.float32
    bf16 = mybir.dt.bfloat16
    fp8 = mybir.dt.float8e4
    AF = mybir.ActivationFunctionType
    DR = mybir.MatmulPerfMode.DoubleRow

    nc = bacc.Bacc("TRN2", target_bir_lowering=False, debug=False,
                   num_devices=NCORES)

    def din(name, shape, dt=bf16):
        return nc.dram_tensor(name, shape, dt, kind="ExternalInput").ap()

    xt_enc = din("xt_enc", [H, SRC * B])
    xt_dec = din("xt_dec", [H, SB])
    wi_e = din("wi_e", [KC, 128, 4 * H])
    wh_e = din("wh_e", [KC, 128, 4 * H])
    wi_d = din("wi_d", [KC, 128, 4 * H])
    wh_d = din("wh_d", [KC, 128, 4 * H])
    bias_e = din("bias_e", [128, NG], f32)
    bias_d = din("bias_d", [128, NG], f32)
    mask_in = din("mask", [SRC, 128, KC * B], mybir.dt.uint8)
    ident_in = din("ident", [128, 128])
    wot_in = din("wot", [KC, 128, VSP], fp8)
    bout_in = din("bout_sc", [128, VSP])
    wtgt_in = din("wtgt", [KC, 128, SB], fp8)

    out_s = nc.dram_tensor("out_s", [128, SBC], f32, kind="ExternalOutput").ap()
    out_l = nc.dram_tensor("out_l", [1, SB], f32, kind="ExternalOutput").ap()

    with tile.TileContext(nc) as tc:
        from contextlib import ExitStack
        with ExitStack() as ctx:
            # ---- pools ----
            pconst = ctx.enter_context(tc.tile_pool(name="const", bufs=1))
            pht = ctx.enter_context(tc.tile_pool(name="ht", bufs=1))
            pgx = ctx.enter_context(tc.tile_pool(name="gx", bufs=2))
            pw = ctx.enter_context(tc.tile_pool(name="w", bufs=1))
            pwt = ctx.enter_context(tc.tile_pool(name="wt", bufs=1))
            pwo = ctx.enter_context(tc.tile_pool(name="wo", bufs=1))
            pxt = ctx.enter_context(tc.tile_pool(name="xtw", bufs=2))
            pstate = ctx.enter_context(tc.tile_pool(name="state", bufs=3))
            pact = ctx.enter_context(tc.tile_pool(name="act", bufs=2))
            pmask = ctx.enter_context(tc.tile_pool(name="mask", bufs=2))
            plog = ctx.enter_context(tc.tile_pool(name="log", bufs=2))
            pfp = ctx.enter_context(tc.tile_pool(name="fp", bufs=4))

            # ---- constants ----
            def dve_const(src_ap, shape, dt, tag):
                dma_t = pconst.tile(shape, dt, tag=f"{tag}_dma")
                nc.sync.dma_start(dma_t[:], src_ap)
                t = pconst.tile(shape, dt, tag=tag)
                nc.vector.tensor_copy(t[:], dma_t[:])
                return t

            bias_e_t = dve_const(bias_e[:], [128, NG], f32, "be")
            bias_d_t = dve_const(bias_d[:], [128, NG], f32, "bd")
            ones_t = pconst.tile([128, 1], f32)
            nc.vector.memset(ones_t[:], 1.0)
            ones_row = pconst.tile([1, 128], bf16)
            nc.vector.memset(ones_row[:], 1.0)
            ident = pconst.tile([128, 128], bf16)
            nc.sync.dma_start(ident[:], ident_in[:])

            # deferred-prefetch tiles (DMAs issued on the ACT engine's DMA
            # queue at step 2 so they don't block sync-queue DMAs)
            wtg_tiles = [pwt.tile([128, 2 * SB], fp8, tag=f"wtg{h}",
                                  name=f"wtg{h}") for h in range(2)]
            wtg_s = lambda k: wtg_tiles[k // 2][:, (k % 2) * SB:
                                                (k % 2 + 1) * SB]
            wof_tiles = [pwo.tile([128, 2 * VSP], fp8, tag=f"wof{h}",
                                  name=f"wof{h}") for h in range(2)]
            bout = pconst.tile([128, VSP], bf16)

            def load_w(dram, pool, tag, eng, width=4 * H, eng2=None):
                ts = []
                dw = dram.shape[2]
                for k in range(KC):
                    t = pool.tile([128, width], bf16, tag=f"{tag}{k}")
                    e = eng if (eng2 is None or k < 2) else eng2
                    e.dma_start(t[:, :dw], dram[k])
                    ts.append(t)
                return ts

            wd = {}

            def prefetch_all():
                wd["i"] = load_w(wi_d, pw, "wid", nc.scalar)
                wd["h"] = load_w(wh_d, pw, "whd", nc.scalar)
                for half in range(2):
                    for j in range(2):
                        nc.scalar.dma_start(
                            wtg_tiles[half][:, j * SB:(j + 1) * SB],
                            wtgt_in[half * 2 + j])
                        nc.scalar.dma_start(
                            wof_tiles[half][:, j * VSP:(j + 1) * VSP],
                            wot_in[half * 2 + j])
                nc.scalar.dma_start(bout[:], bout_in[:])

            # HT: decoder hidden states, transposed, col = k*SBP + t*64 + b
            ht = pht.tile([128, KC * SBP], bf16)
            nc.vector.memset(ht[:], 0.0)

            we_i = load_w(wi_e, pw, "wie", nc.sync, eng2=nc.scalar)
            we_h = None   # loaded after the prologue window's xt DMA

            l_sb = pconst.tile([1, SB], f32)

            # ============ unified 95-step recurrence ============
            with (
                tc.tile_pool(name="psA", bufs=3, space=bass.MemorySpace.PSUM)
                    as psA,
                tc.tile_pool(name="psB", bufs=2, space=bass.MemorySpace.PSUM)
                    as psB,
                tc.tile_pool(name="psC", bufs=2, space=bass.MemorySpace.PSUM)
                    as psC,
                tc.tile_pool(name="psT", bufs=1, space=bass.MemorySpace.PSUM)
                    as psT,
            ):
                def bulk_pieces(xt_src, wkey, bias_t, t0, nsteps):
                    """Yield closures: piece 0 = DMA + gx alloc, one piece
                    per gate chunk (4 MMs + bias fold to gx), then a
                    sentinel returning the gx tile."""
                    wi_t = we_i if wkey == "enc" else wd["i"]
                    w = nsteps * B
                    state = {}

                    def p_dma():
                        state["gx"] = pgx.tile([128, NG * WIN * B], bf16,
                                               tag="gx", name="gxw")
                        xtw = []
                        for k in range(KC):
                            t = pxt.tile([128, WIN * B], bf16, tag=f"xt{k}")
                            nc.sync.dma_start(
                                t[:, :w], xt_src[k * 128:(k + 1) * 128,
                                                 t0 * B:t0 * B + w])
                            xtw.append(t)
                        state["xtw"] = xtw
                    yield p_dma

                    def mk_chunk(g):
                        def p_mm():
                            pb = psA.tile([128, 512], f32, tag="psA")
                            for k in range(KC):
                                nc.tensor.matmul(
                                    pb[:, :w],
                                    wi_t[k][:, g * 128:(g + 1) * 128],
                                    state["xtw"][k][:, :w],
                                    start=(k == 0), stop=(k == KC - 1))
                            gxs = state["gx"][:, g * WIN * B:g * WIN * B + w]

                            def p_bias():
                                # deferred to the next step's top so the
                                # 700ns bias op never sits in the engine
                                # FIFO ahead of chain ops it would block
                                if g % 2 == 0:
                                    nc.vector.tensor_scalar_add(
                                        gxs, pb[:, :w], bias_t[:, g:g + 1])
                                else:
                                    nc.scalar.activation(
                                        gxs, pb[:, :w], AF.Identity,
                                        bias=bias_t[:, g:g + 1])
                            return p_bias
                        return p_mm
                    for g in range(NG):
                        yield mk_chunk(g)
                    yield lambda: state["gx"]

                def tgt_piece(nt):
                    """One 512-col piece of the gathered-target logits:
                    l[s] = sum_h ht[h,s]*wtgt[h,s] (DVE/Pool muls + ones
                    contraction), scaled back from the fp8 W scale."""
                    wdt = min(512, SB - nt * 512)
                    pt = psT.tile([128, 512], f32, tag="psT")
                    for k in range(KC):
                        prod = plog.tile([128, 512], f32, tag=f"prod{k % 2}",
                                         name="prod")
                        eng = nc.vector if k % 2 == 0 else nc.gpsimd
                        eng.tensor_mul(
                            prod[:, :wdt],
                            ht[:, k * SBP + nt * 512:k * SBP + nt * 512 + wdt],
                            wtg_s(k)[:, nt * 512:nt * 512 + wdt])
                        nc.tensor.matmul(pt[0:1, :wdt], ones_t[:],
                                         prod[:, :wdt],
                                         start=(k == 0), stop=(k == KC - 1))
                    nc.scalar.activation(l_sb[:, nt * 512:nt * 512 + wdt],
                                         pt[0:1, :wdt], AF.Copy,
                                         scale=1.0 / W_SC)

                def lstm_step(gx, lt, h_rhs, c_prev, wh_t, out01, out23,
                              split_sig):
                    pA = psA.tile([128, 512], f32, tag="psA")  # i|f
                    pB = psB.tile([128, 256], f32, tag="psB")  # g
                    pC = psC.tile([128, 256], f32, tag="psC")  # o

                    gx_r = gx[:].rearrange("p (g s) -> p g s", g=NG)
                    nc.tensor.matmul(
                        pB[:].rearrange("p (g s) -> p g s", g=4),
                        ident[:], gx_r[:, 12:16, lt * B:(lt + 1) * B],
                        start=True, stop=False)
                    nc.tensor.matmul(
                        pA[:].rearrange("p (g s) -> p g s", g=8),
                        ident[:], gx_r[:, 0:8, lt * B:(lt + 1) * B],
                        start=True, stop=False)
                    nc.tensor.matmul(
                        pC[:].rearrange("p (g s) -> p g s", g=4),
                        ident[:], gx_r[:, 8:12, lt * B:(lt + 1) * B],
                        start=True, stop=False)

                    def mm(c, k, stop):
                        if c in G_CH:
                            dst = pB[:, (c - 12) * B:(c - 11) * B]
                        elif c in O_CH:
                            dst = pC[:, (c - 8) * B:(c - 7) * B]
                        else:
                            dst = pA[:, c * B:(c + 1) * B]
                        nc.tensor.matmul(dst,
                                         wh_t[k][:, c * 128:(c + 1) * 128],
                                         h_rhs(k), start=False, stop=stop)

                    # G group first (k01 before k23: h halves arrive
                    # staggered from the previous step's split tail)
                    for k in (0, 1):
                        for c in G_CH:
                            mm(c, k, False)
                    for k in (2, 3):
                        for c in G_CH:
                            mm(c, k, (k == 3 and c == 15))
                    tng = pact.tile([128, 256], bf16, tag="tng")
                    nc.scalar.activation(tng[:], pB[:], AF.Tanh)
                    # F group
                    for c in F_CH:
                        for k in range(KC):
                            mm(c, k, False)
                    sig = pact.tile([128, 512], bf16, tag="sig")
                    if split_sig:
                        nc.scalar.activation(sig[:, 256:512], pA[:, 256:512],
                                             AF.Sigmoid)
                        t1 = pact.tile([128, 256], bf16, tag="t1")
                        nc.gpsimd.tensor_mul(t1[:], sig[:, 256:512],
                                             c_prev[:])
                    # I group
                    for c in I_CH:
                        for k in range(KC):
                            mm(c, k, (c == 3 and k == 3))
                    if split_sig:
                        nc.scalar.activation(sig[:, 0:256], pA[:, 0:256],
                                             AF.Sigmoid)
                    else:
                        nc.scalar.activation(sig[:], pA[:], AF.Sigmoid)
                        t1 = pact.tile([128, 256], bf16, tag="t1")
                        nc.gpsimd.tensor_mul(t1[:], sig[:, 256:512],
                                             c_prev[:])
                    t2 = pact.tile([128, 256], bf16, tag="t2")
                    nc.vector.tensor_mul(t2[:], sig[:, 0:256], tng[:])
                    # O group
                    for c in O_CH:
                        for k in range(KC):
                            mm(c, k, (c == 11 and k == 3))
                    c_new = pstate.tile([128, 256], bf16, tag="c")
                    nc.vector.tensor_add(c_new[:], t1[:], t2[:])
                    sgo = pact.tile([128, 256], bf16, tag="sgo")
                    nc.scalar.activation(sgo[:], pC[:], AF.Sigmoid)
                    tnc = pact.tile([128, 256], bf16, tag="tnc")
                    nc.scalar.activation(tnc[:], c_new[:], AF.Tanh)
                    # h = sgo*tnc: halves on DVE (k01) and Pool (k23)
                    nc.vector.tensor_mul(out01, sgo[:, 0:128], tnc[:, 0:128])
                    nc.gpsimd.tensor_mul(out23, sgo[:, 128:256],
                                         tnc[:, 128:256])
                    return c_new

                h_prev = pstate.tile([128, KC * B], bf16, tag="h")
                nc.vector.memset(h_prev[:], 0.0)
                c_prev = pstate.tile([128, 256], bf16, tag="c")
                nc.vector.memset(c_prev[:], 0.0)

                win_list = (
                    [(xt_enc, "enc", bias_e_t, t0, min(WIN, SRC - t0))
                     for t0 in range(0, SRC, WIN)] +
                    [(xt_dec, "dec", bias_d_t, t0, min(WIN, DEC - t0))
                     for t0 in range(0, DEC, WIN)])

                pending = []

                def run_piece(p):
                    r = p()
                    if callable(r):
                        pending.append(r)
                        return None
                    return r

                def flush_pending():
                    for b in pending:
                        b()
                    pending.clear()

                gx = None
                pro_gen = bulk_pieces(*win_list[0])   # prologue window
                next(pro_gen)()                       # xt DMA first
                we_h = load_w(wh_e, pw, "whe", nc.sync, eng2=nc.scalar)
                for p in pro_gen:
                    r = run_piece(p)
                    gx = r if r is not None else gx
                flush_pending()
                next_idx = 1
                next_gen = bulk_pieces(*win_list[next_idx])
                gx_next = None

                ht_r = ht[:].rearrange("p (k s) -> p k s", k=KC)
                step_no = 0
                for phase, nsteps in (("enc", SRC), ("dec", DEC)):
                    wh_t = we_h if phase == "enc" else wd["h"]
                    for t in range(nsteps):
                        if t % WIN == 0 and step_no > 0:
                            while next_gen is not None:
                                try:
                                    p = next(next_gen)
                                except StopIteration:
                                    next_gen = None
                                    break
                                r = run_piece(p)
                                gx_next = r if r is not None else gx_next
                            flush_pending()
                            gx, gx_next = gx_next, None
                            next_idx += 1
                            if next_idx < len(win_list):
                                next_gen = bulk_pieces(*win_list[next_idx])
                        else:
                            flush_pending()
                        if phase == "dec" and t % WIN == 0 and t > 0:
                            tgt_piece(t // WIN - 1)
                        if phase == "enc" or t == 0:
                            hp = h_prev
                            rhs = (lambda k, hp=hp:
                                   hp[:, k * B:(k + 1) * B])
                        else:
                            rhs = (lambda k, tp=t - 1:
                                   ht[:, k * SBP + tp * B:
                                      k * SBP + (tp + 1) * B])
                        if phase == "enc":
                            h_new = pstate.tile([128, KC * B], bf16, tag="h")
                            out01 = h_new[:, 0:128]
                            out23 = h_new[:, 128:256]
                        else:
                            out01 = ht_r[:, 0:2, t * B:(t + 1) * B]
                            out23 = ht_r[:, 2:4, t * B:(t + 1) * B]
                        c_new = lstm_step(gx, t % WIN, rhs, c_prev, wh_t,
                                          out01, out23,
                                          split_sig=(phase == "dec"))
                        if phase == "enc":
                            mk = pmask.tile([128, KC * B], mybir.dt.uint8,
                                            tag="mk")
                            nc.sync.dma_start(mk[:], mask_in[t])
                            nc.vector.copy_predicated(h_new[:], mk[:],
                                                      h_prev[:])
                            nc.vector.copy_predicated(c_new[:], mk[:],
                                                      c_prev[:])
                            h_prev = h_new
                        c_prev = c_new
                        step_no += 1
                        if step_no == 2:
                            prefetch_all()
                        if next_gen is not None:
                            for _ in range(2):
                                try:
                                    p = next(next_gen)
                                except StopIteration:
                                    next_gen = None
                                    break
                                r = run_piece(p)
                                gx_next = r if r is not None else gx_next
                flush_pending()
                tgt_piece(5)
                nc.sync.dma_start(out_l[:], l_sb[:])

            # ============ fp8 DoubleRow vocab logits + sum-exp ============
            with tc.tile_pool(name="psL", bufs=2,
                              space=bass.MemorySpace.PSUM) as psL:
                s_all = pconst.tile([128, SBC], f32)

                for sb in range(SBC):
                    st = pfp.tile([128, 512], fp8, tag="st")
                    nc.vector.tensor_scalar_mul(
                        st[:].rearrange("p (k s) -> p k s", k=KC),
                        ht_r[:, :, sb * 128:(sb + 1) * 128], H_SC)
                    sh = []
                    for half in range(2):
                        pl = psL.tile([128, 2048], f32, tag="psL")
                        pe_bias = (half == 0)
                        if pe_bias:
                            for v in range(4):
                                col = half * 2048 + v * 512
                                nc.tensor.matmul(
                                    pl[:, v * 512:(v + 1) * 512],
                                    ones_row[:], bout[0:1, col:col + 512],
                                    start=True, stop=False)
                        for pair in range(2):
                            lhs = st[:, pair * 256:(pair + 1) * 256]\
                                .rearrange("p (two m) -> p two m", two=2)
                            for v in range(4):
                                col = half * 2048 + v * 512
                                rhs = wof_tiles[pair][:]\
                                    .rearrange("p (two v) -> p two v", two=2)\
                                    [:, :, col:col + 512]
                                nc.tensor.matmul(
                                    pl[:, v * 512:(v + 1) * 512], lhs, rhs,
                                    start=(not pe_bias and pair == 0),
                                    stop=(pair == 1), perf_mode=DR)
                        if not pe_bias:
                            nc.vector.tensor_add(
                                pl[:], pl[:],
                                bout[:, half * 2048:half * 2048 + 2048])
                        sh_t = plog.tile([128, 1], f32, tag=f"sh{half}",
                                         name="sh_t")
                        nc.scalar.activation(pl[:], pl[:], AF.Exp,
                                             scale=1.0 / L_SC,
                                             accum_out=sh_t[:])
                        sh.append(sh_t)
                    nc.gpsimd.tensor_add(s_all[:, sb:sb + 1],
                                         sh[0][:], sh[1][:])
                nc.sync.dma_start(out_s[:], s_all[:])

    nc.compile()
    return nc


def _prep(inputs):
    """Host-side data prep. Returns per-core in_maps + host combine data."""
    il = np.asarray(inputs["input_lines"])
    tl = np.asarray(inputs["target_lines"])
    f = lambda k: np.asarray(inputs[k], np.float32)
    emb_in, emb_tgt = f("emb_in").copy(), f("emb_tgt").copy()
    emb_in[0] = 0.0
    emb_tgt[0] = 0.0
    W_out, b_out = f("W_out"), f("b_out")

    perm = np.concatenate([np.arange(0, 512), np.arange(512, 1024),
                           np.arange(1536, 2048), np.arange(1024, 1536)])

    def wt(w):  # [2048,512] -> [4,128,2048] bf16 (transposed, gate-permuted)
        return np.ascontiguousarray(
            w[perm].T.reshape(KC, 128, 4 * H)).astype(BF16)

    def wt_pair(w):  # -> [2,128,2*2048] fp8 (k-pairs side by side, scaled)
        r = np.clip(w[perm].T * WI_SC, -240, 240).reshape(KC, 128, 4 * H)
        pairs = np.stack([
            np.concatenate([r[0], r[1]], axis=1),
            np.concatenate([r[2], r[3]], axis=1)])
        return np.ascontiguousarray(pairs).astype(FP8)

    def bias(bi, bh):  # -> [128, 16] f32
        return np.ascontiguousarray(
            (bi + bh)[perm].reshape(NG, 128).T).astype(np.float32)

    x_enc = emb_in[il.reshape(-1)]                       # [3072, 512]
    xt_enc = np.ascontiguousarray(x_enc.T).astype(BF16)  # [512, 3072]
    tgt_in = tl[:DEC].reshape(-1)
    x_dec = emb_tgt[tgt_in]
    xt_dec = np.ascontiguousarray(x_dec.T).astype(BF16)  # [512, 3008]

    m = (il == 0).astype(np.uint8)                       # [48, 64]
    mask = np.ascontiguousarray(np.broadcast_to(
        m[:, None, None, :], (SRC, 128, KC, B)).reshape(
            SRC, 128, KC * B)).astype(np.uint8)

    tgt_next = tl[1:TGT].reshape(-1)                     # [3008]
    wtgt = np.ascontiguousarray(np.clip(
        W_out[tgt_next].T * W_SC, -240, 240).reshape(
            KC, 128, SB)).astype(FP8)
    b_tgt = b_out[tgt_next].astype(np.float64)

    common = dict(
        xt_enc=xt_enc, xt_dec=xt_dec,
        wi_e=wt(f("W_ih_e")), wh_e=wt(f("W_hh_e")),
        wi_d=wt(f("W_ih_d")), wh_d=wt(f("W_hh_d")),
        bias_e=bias(f("b_ih_e"), f("b_hh_e")),
        bias_d=bias(f("b_ih_d"), f("b_hh_d")),
        mask=mask, wtgt=wtgt,
        ident=np.eye(128, dtype=BF16),
    )
    in_maps = []
    for c in range(NCORES):
        ws = np.zeros((VSP, H), np.float32)
        ws[:VSH] = W_out[c * VSH:(c + 1) * VSH]
        bs = np.full(VSP, -88.0 * L_SC, np.float32)
        bs[:VSH] = b_out[c * VSH:(c + 1) * VSH] * L_SC
        in_maps.append(dict(
            common,
            wot=np.ascontiguousarray(np.clip(
                ws.T * W_SC, -240, 240).reshape(KC, 128, VSP)).astype(FP8),
            bout_sc=np.ascontiguousarray(
                np.broadcast_to(bs, (128, VSP))).astype(BF16),
        ))
    return in_maps, b_tgt


def _combine(results, b_tgt):
    s = np.zeros(SBP, np.float64)
    for r in results:
        s += np.asarray(r["out_s"], np.float64).T.reshape(-1)
    s = s[:SB]
    lse = np.log(s)
    l_tgt = np.asarray(results[0]["out_l"], np.float64).reshape(-1) + b_tgt
    return np.float32((lse - l_tgt).sum() / B)


def kernel(**inputs):
    global _COMPILED
    from concourse.bass_utils import run_bass_kernel_spmd
    in_maps, b_tgt = _prep(inputs)
    if _COMPILED is None:
        _COMPILED = _build()
    res = run_bass_kernel_spmd(_COMPILED, in_maps, list(range(NCORES)))
    return _combine(res.results, b_tgt)


if __name__ == "__main__":
    import reference
    inp = reference.setup_inputs()
    expected = np.asarray(reference.reference(**inp))
    actual = kernel(**{k: np.asarray(v) for k, v in inp.items()})
    err = abs(actual - expected) / max(abs(expected), 1e-9)
    print(f"expected={expected} actual={actual} rel_err={err:.3e}")


# revision 16
# speedup vs baseline: 1.0499x; 1.0494x over previous
"""Encoder-decoder LSTM seq2seq loss kernel for 8 TRN2 NeuronCores.

Strategy:
  - LSTM recurrences (encoder 48 steps, decoder 47 steps) are replicated on
    every core in gate-major layout: gates^T [2048, 64] computed as 16
    [128,64] PSUM chunks, state kept transposed (hT [128, 4*64]) so no
    per-step transposes are needed.
  - Input-side gate contributions (x @ W_ih^T + b) are batched in 8-step
    windows as full-utilization [128,128]x[128,512] matmuls interleaved
    between recurrence steps; the per-step x-injection into the gates
    PSUM is an identity matmul on the PE (exact for 1.0*bf16).
  - The per-step chain: gate groups issue G, F, I, O; the decoder splits
    the i/f sigmoid so t1 = sig_f*c (Pool) and t2 = sig_i*tanh_g (DVE)
    start while the O matmuls stream; the h = sig_o*tanh(c) tail is
    split across DVE (k01) and Pool (k23) so the next step's k01
    matmuls start early. The encoder (more DVE load: masks) keeps a
    merged i|f sigmoid and whole-width predicated restores.
  - All heavy prefetches (decoder weights, fp8 W_out shard, fp8
    gathered-target W_out, scaled output bias) are issued on the ACT
    engine's separate DMA queue two steps into the recurrence, so they
    neither delay the recurrence-critical startup DMAs nor block the
    per-step mask/window DMAs on the sync queue.
  - Target logits are computed incrementally during the decoder (one
    512-column piece per 8 steps: DVE/Pool elementwise muls + ones
    contraction on the PE) so no separate phase remains.
  - The 32k-vocab logits GEMM runs as fp8 DoubleRow matmuls (2x fewer
    PE cycles): ht is converted per sb-chunk to scaled fp8 (x32), W_out
    shard is host-scaled fp8 (x256); the output bias (x8192) is
    injected by K=1 ones-matmuls as the PSUM accumulation start, and
    the 1/8192 rescale is folded into the ACT Exp's scale, whose
    free-axis accumulator produces the softmax denominator directly.
    Per [128,2048] PSUM half: 4 bias MMs + 8 DoubleRow MMs -> Exp.
  - Host combines per-core partial sum-exp + target logits into the
    scalar loss (tiny: 8 x [128,24] + [1,3008]).
"""

import sys

sys.path.insert(0, "/opt/trn_rl_repo")

import numpy as np
import ml_dtypes

BF16 = ml_dtypes.bfloat16
FP8 = ml_dtypes.float8_e4m3

# Model dims (hardcoded per contract)
SRC, TGT, B, H, V = 48, 48, 64, 512, 32000
DEC = TGT - 1                  # 47 decoder steps
SB = DEC * B                   # 3008 (step*batch)
SBC = 24                       # ceil(3008/128) sb-chunks
SBP = SBC * 128                # 3072 padded
NCORES = 8
VSH = V // NCORES              # 4000 vocab rows per core
VSP = 4096                     # padded shard
WIN = 8                        # bulk x-part window (steps)
NG = 16                        # gate chunks (2048/128)
KC = 4                         # hidden chunks (512/128)

H_SC = 32.0                    # fp8 scale for ht
XE_SC = 256.0                  # fp8 scale for x embeddings
WI_SC = 256.0                  # fp8 scale for W_ih
X_SC = XE_SC * WI_SC           # x-part gate rescale (65536)
W_SC = 256.0                   # fp8 scale for W_out / W_out[tgt]
L_SC = H_SC * W_SC             # logits scale (8192)

# gate-chunk indices in the permuted [i f o g] weight layout
I_CH = list(range(0, 4))
F_CH = list(range(4, 8))
O_CH = list(range(8, 12))
G_CH = list(range(12, 16))

_COMPILED = None


def _build():
    import concourse.bass as bass
    import concourse.bacc as bacc
    import concourse.tile as tile
    from concourse import mybir

    f32 = mybir.dt.float32
    bf16 = mybir.dt.bfloat16
    fp8 = mybir.dt.float8e4
    AF = mybir.ActivationFunctionType
    DR = mybir.MatmulPerfMode.DoubleRow

    nc = bacc.Bacc("TRN2", target_bir_lowering=False, debug=False,
                   num_devices=NCORES)

    def din(name, shape, dt=bf16):
        return nc.dram_tensor(name, shape, dt, kind="ExternalInput").ap()

    xt_enc = din("xt_enc", [H, SRC * B])
    xt_dec = din("xt_dec", [H, SB])
    wi_e = din("wi_e", [KC, 128, 4 * H])
    wh_e = din("wh_e", [KC, 128, 4 * H])
    wi_d = din("wi_d", [KC, 128, 4 * H])
    wh_d = din("wh_d", [KC, 128, 4 * H])
    bias_e = din("bias_e", [128, NG], f32)
    bias_d = din("bias_d", [128, NG], f32)
    mask_in = din("mask", [SRC, 128, KC * B], mybir.dt.uint8)
    ident_in = din("ident", [128, 128])
    wot_in = din("wot", [KC, 128, VSP], fp8)
    bout_in = din("bout_sc", [128, VSP])
    wtgt_in = din("wtgt", [KC, 128, SB], fp8)

    out_s = nc.dram_tensor("out_s", [128, SBC], f32, kind="ExternalOutput").ap()
    out_l = nc.dram_tensor("out_l", [1, SB], f32, kind="ExternalOutput").ap()

    with tile.TileContext(nc) as tc:
        from contextlib import ExitStack
        with ExitStack() as ctx:
            # ---- pools ----
            pconst = ctx.enter_context(tc.tile_pool(name="const", bufs=1))
            pht = ctx.enter_context(tc.tile_pool(name="ht", bufs=1))
            pgx = ctx.enter_context(tc.tile_pool(name="gx", bufs=2))
            pw = ctx.enter_context(tc.tile_pool(name="w", bufs=1))
            pwt = ctx.enter_context(tc.tile_pool(name="wt", bufs=1))
            pwo = ctx.enter_context(tc.tile_pool(name="wo", bufs=1))
            pxt = ctx.enter_context(tc.tile_pool(name="xtw", bufs=2))
            pstate = ctx.enter_context(tc.tile_pool(name="state", bufs=3))
            pact = ctx.enter_context(tc.tile_pool(name="act", bufs=2))
            pmask = ctx.enter_context(tc.tile_pool(name="mask", bufs=2))
            plog = ctx.enter_context(tc.tile_pool(name="log", bufs=2))
            pfp = ctx.enter_context(tc.tile_pool(name="fp", bufs=4))

            # ---- constants ----
            def dve_const(src_ap, shape, dt, tag):
                dma_t = pconst.tile(shape, dt, tag=f"{tag}_dma")
                nc.sync.dma_start(dma_t[:], src_ap)
                t = pconst.tile(shape, dt, tag=tag)
                nc.vector.tensor_copy(t[:], dma_t[:])
                return t

            bias_e_t = dve_const(bias_e[:], [128, NG], f32, "be")
            bias_d_t = dve_const(bias_d[:], [128, NG], f32, "bd")
            ones_t = pconst.tile([128, 1], f32)
            nc.vector.memset(ones_t[:], 1.0)
            ones_row = pconst.tile([1, 128], bf16)
            nc.vector.memset(ones_row[:], 1.0)
            ident = pconst.tile([128, 128], bf16)
            nc.sync.dma_start(ident[:], ident_in[:])

            # deferred-prefetch tiles (DMAs issued on the ACT engine's DMA
            # queue at step 2 so they don't block sync-queue DMAs)
            wtg_tiles = [pwt.tile([128, 2 * SB], fp8, tag=f"wtg{h}",
                                  name=f"wtg{h}") for h in range(2)]
            wtg_s = lambda k: wtg_tiles[k // 2][:, (k % 2) * SB:
                                                (k % 2 + 1) * SB]
            wof_tiles = [pwo.tile([128, 2 * VSP], fp8, tag=f"wof{h}",
                                  name=f"wof{h}") for h in range(2)]
            bout = pconst.tile([128, VSP], bf16)

            def load_w(dram, pool, tag, eng, width=4 * H, eng2=None):
                ts = []
                dw = dram.shape[2]
                for k in range(KC):
                    t = pool.tile([128, width], bf16, tag=f"{tag}{k}")
                    e = eng if (eng2 is None or k < 2) else eng2
                    e.dma_start(t[:, :dw], dram[k])
                    ts.append(t)
                return ts

            wd = {}

            def prefetch_all():
                wd["i"] = load_w(wi_d, pw, "wid", nc.scalar)
                wd["h"] = load_w(wh_d, pw, "whd", nc.scalar)
                for half in range(2):
                    for j in range(2):
                        nc.scalar.dma_start(
                            wtg_tiles[half][:, j * SB:(j + 1) * SB],
                            wtgt_in[half * 2 + j])
                        nc.scalar.dma_start(
                            wof_tiles[half][:, j * VSP:(j + 1) * VSP],
                            wot_in[half * 2 + j])
                nc.scalar.dma_start(bout[:], bout_in[:])

            # HT: decoder hidden states, transposed, col = k*SBP + t*64 + b
            ht = pht.tile([128, KC * SBP], bf16)
            nc.vector.memset(ht[:], 0.0)

            we_i = load_w(wi_e, pw, "wie", nc.sync, eng2=nc.scalar)
            we_h = None   # loaded after the prologue window's xt DMA

            l_sb = pconst.tile([1, SB], f32)

            # ============ unified 95-step recurrence ============
            with (
                tc.tile_pool(name="psA", bufs=3, space=bass.MemorySpace.PSUM)
                    as psA,
                tc.tile_pool(name="psB", bufs=2, space=bass.MemorySpace.PSUM)
                    as psB,
                tc.tile_pool(name="psC", bufs=2, space=bass.MemorySpace.PSUM)
                    as psC,
                tc.tile_pool(name="psT", bufs=1, space=bass.MemorySpace.PSUM)
                    as psT,
            ):
                def bulk_pieces(xt_src, wkey, bias_t, t0, nsteps):
                    """Yield closures: piece 0 = DMA + gx alloc, one piece
                    per gate chunk (4 MMs + bias fold to gx), then a
                    sentinel returning the gx tile."""
                    wi_t = we_i if wkey == "enc" else wd["i"]
                    w = nsteps * B
                    state = {}

                    def p_dma():
                        state["gx"] = pgx.tile([128, NG * WIN * B], bf16,
                                               tag="gx", name="gxw")
                        xtw = []
                        for k in range(KC):
                            t = pxt.tile([128, WIN * B], bf16, tag=f"xt{k}")
                            nc.sync.dma_start(
                                t[:, :w], xt_src[k * 128:(k + 1) * 128,
                                                 t0 * B:t0 * B + w])
                            xtw.append(t)
                        state["xtw"] = xtw
                    yield p_dma

                    def mk_chunk(g):
                        def p_mm():
                            pb = psA.tile([128, 512], f32, tag="psA")
                            for k in range(KC):
                                nc.tensor.matmul(
                                    pb[:, :w],
                                    wi_t[k][:, g * 128:(g + 1) * 128],
                                    state["xtw"][k][:, :w],
                                    start=(k == 0), stop=(k == KC - 1))
                            gxs = state["gx"][:, g * WIN * B:g * WIN * B + w]

                            def p_bias():
                                # deferred to the next step's top so the
                                # 700ns bias op never sits in the engine
                                # FIFO ahead of chain ops it would block
                                if g % 2 == 0:
                                    nc.vector.tensor_scalar_add(
                                        gxs, pb[:, :w], bias_t[:, g:g + 1])
                                else:
                                    nc.scalar.activation(
                                        gxs, pb[:, :w], AF.Identity,
                                        bias=bias_t[:, g:g + 1])
                            return p_bias
                        return p_mm
                    for g in range(NG):
                        yield mk_chunk(g)
                    yield lambda: state["gx"]

                def tgt_piece(nt):
                    """One 512-col piece of the gathered-target logits:
                    l[s] = sum_h ht[h,s]*wtgt[h,s] (DVE/Pool muls + ones
                    contraction), scaled back from the fp8 W scale."""
                    wdt = min(512, SB - nt * 512)
                    pt = psT.tile([128, 512], f32, tag="psT")
                    for k in range(KC):
                        prod = plog.tile([128, 512], f32, tag=f"prod{k % 2}",
                                         name="prod")
                        eng = nc.vector if k % 2 == 0 else nc.gpsimd
                        eng.tensor_mul(
                            prod[:, :wdt],
                            ht[:, k * SBP + nt * 512:k * SBP + nt * 512 + wdt],
                            wtg_s(k)[:, nt * 512:nt * 512 + wdt])
                        nc.tensor.matmul(pt[0:1, :wdt], ones_t[:],
                                         prod[:, :wdt],
                                         start=(k == 0), stop=(k == KC - 1))
                    nc.scalar.activation(l_sb[:, nt * 512:nt * 512 + wdt],
                                         pt[0:1, :wdt], AF.Copy,
                                         scale=1.0 / W_SC)

                def lstm_step(gx, lt, h_rhs, c_prev, wh_t, out01, out23,
                              split_sig):
                    pA = psA.tile([128, 512], f32, tag="psA")  # i|f
                    pB = psB.tile([128, 256], f32, tag="psB")  # g
                    pC = psC.tile([128, 256], f32, tag="psC")  # o

                    gx_r = gx[:].rearrange("p (g s) -> p g s", g=NG)
                    nc.tensor.matmul(
                        pB[:].rearrange("p (g s) -> p g s", g=4),
                        ident[:], gx_r[:, 12:16, lt * B:(lt + 1) * B],
                        start=True, stop=False)
                    nc.tensor.matmul(
                        pA[:].rearrange("p (g s) -> p g s", g=8),
                        ident[:], gx_r[:, 0:8, lt * B:(lt + 1) * B],
                        start=True, stop=False)
                    nc.tensor.matmul(
                        pC[:].rearrange("p (g s) -> p g s", g=4),
                        ident[:], gx_r[:, 8:12, lt * B:(lt + 1) * B],
                        start=True, stop=False)

                    def mm(c, k, stop):
                        if c in G_CH:
                            dst = pB[:, (c - 12) * B:(c - 11) * B]
                        elif c in O_CH:
                            dst = pC[:, (c - 8) * B:(c - 7) * B]
                        else:
                            dst = pA[:, c * B:(c + 1) * B]
                        nc.tensor.matmul(dst,
                                         wh_t[k][:, c * 128:(c + 1) * 128],
                                         h_rhs(k), start=False, stop=stop)

                    # G group first (k01 before k23: h halves arrive
                    # staggered from the previous step's split tail)
                    for k in (0, 1):
                        for c in G_CH:
                            mm(c, k, False)
                    for k in (2, 3):
                        for c in G_CH:
                            mm(c, k, (k == 3 and c == 15))
                    tng = pact.tile([128, 256], bf16, tag="tng")
                    nc.scalar.activation(tng[:], pB[:], AF.Tanh)
                    # F group
                    for c in F_CH:
                        for k in range(KC):
                            mm(c, k, False)
                    sig = pact.tile([128, 512], bf16, tag="sig")
                    if split_sig:
                        nc.scalar.activation(sig[:, 256:512], pA[:, 256:512],
                                             AF.Sigmoid)
                        t1 = pact.tile([128, 256], bf16, tag="t1")
                        nc.gpsimd.tensor_mul(t1[:], sig[:, 256:512],
                                             c_prev[:])
                    # I group
                    for c in I_CH:
                        for k in range(KC):
                            mm(c, k, (c == 3 and k == 3))
                    if split_sig:
                        nc.scalar.activation(sig[:, 0:256], pA[:, 0:256],
                                             AF.Sigmoid)
                    else:
                        nc.scalar.activation(sig[:], pA[:], AF.Sigmoid)
                        t1 = pact.tile([128, 256], bf16, tag="t1")
                        nc.gpsimd.tensor_mul(t1[:], sig[:, 256:512],
                                             c_prev[:])
                    t2 = pact.tile([128, 256], bf16, tag="t2")
                    nc.vector.tensor_mul(t2[:], sig[:, 0:256], tng[:])
                    # O group
                    for c in O_CH:
                        for k in range(KC):
                            mm(c, k, (c == 11 and k == 3))
                    c_new = pstate.tile([128, 256], bf16, tag="c")
                    nc.vector.tensor_add(c_new[:], t1[:], t2[:])
                    sgo = pact.tile([128, 256], bf16, tag="sgo")
                    nc.scalar.activation(sgo[:], pC[:], AF.Sigmoid)
                    tnc = pact.tile([128, 256], bf16, tag="tnc")
                    nc.scalar.activation(tnc[:], c_new[:], AF.Tanh)
                    # h = sgo*tnc: halves on DVE (k01) and Pool (k23)
                    nc.vector.tensor_mul(out01, sgo[:, 0:128], tnc[:, 0:128])
                    nc.gpsimd.tensor_mul(out23, sgo[:, 128:256],
                                         tnc[:, 128:256])
                    return c_new

                h_prev = pstate.tile([128, KC * B], bf16, tag="h")
                nc.vector.memset(h_prev[:], 0.0)
                c_prev = pstate.tile([128, 256], bf16, tag="c")
                nc.vector.memset(c_prev[:], 0.0)

                win_list = (
                    [(xt_enc, "enc", bias_e_t, t0, min(WIN, SRC - t0))
                     for t0 in range(0, SRC, WIN)] +
                    [(xt_dec, "dec", bias_d_t, t0, min(WIN, DEC - t0))
                     for t0 in range(0, DEC, WIN)])

                pending = []

                def run_piece(p):
                    r = p()
                    if callable(r):
                        pending.append(r)
                        return None
                    return r

                def flush_pending():
                    for b in pending:
                        b()
                    pending.clear()

                gx = None
                pro_gen = bulk_pieces(*win_list[0])   # prologue window
                next(pro_gen)()                       # xt DMA first
                we_h = load_w(wh_e, pw, "whe", nc.sync, eng2=nc.scalar)
                for p in pro_gen:
                    r = run_piece(p)
                    gx = r if r is not None else gx
                flush_pending()
                next_idx = 1
                next_gen = bulk_pieces(*win_list[next_idx])
                gx_next = None

                ht_r = ht[:].rearrange("p (k s) -> p k s", k=KC)
                step_no = 0
                for phase, nsteps in (("enc", SRC), ("dec", DEC)):
                    wh_t = we_h if phase == "enc" else wd["h"]
                    for t in range(nsteps):
                        if t % WIN == 0 and step_no > 0:
                            while next_gen is not None:
                                try:
                                    p = next(next_gen)
                                except StopIteration:
                                    next_gen = None
                                    break
                                r = run_piece(p)
                                gx_next = r if r is not None else gx_next
                            flush_pending()
                            gx, gx_next = gx_next, None
                            next_idx += 1
                            if next_idx < len(win_list):
                                next_gen = bulk_pieces(*win_list[next_idx])
                        else:
                            flush_pending()
                        if phase == "dec" and t % WIN == 0 and t > 0:
                            tgt_piece(t // WIN - 1)
                        if phase == "enc" or t == 0:
                            hp = h_prev
                            rhs = (lambda k, hp=hp:
                                   hp[:, k * B:(k + 1) * B])
                        else:
                            rhs = (lambda k, tp=t - 1:
                                   ht[:, k * SBP + tp * B:
                                      k * SBP + (tp + 1) * B])
                        if phase == "enc":
                            h_new = pstate.tile([128, KC * B], bf16, tag="h")
                            out01 = h_new[:, 0:128]
                            out23 = h_new[:, 128:256]
                        else:
                            out01 = ht_r[:, 0:2, t * B:(t + 1) * B]
                            out23 = ht_r[:, 2:4, t * B:(t + 1) * B]
                        c_new = lstm_step(gx, t % WIN, rhs, c_prev, wh_t,
                                          out01, out23,
                                          split_sig=(phase == "dec"))
                        if phase == "enc":
                            mk = pmask.tile([128, KC * B], mybir.dt.uint8,
                                            tag="mk")
                            nc.sync.dma_start(mk[:], mask_in[t])
                            nc.vector.copy_predicated(h_new[:], mk[:],
                                                      h_prev[:])
                            nc.vector.copy_predicated(c_new[:], mk[:],
                                                      c_prev[:])
                            h_prev = h_new
                        c_prev = c_new
                        step_no += 1
                        if step_no == 2:
                            prefetch_all()
                        if next_gen is not None:
                            for _ in range(2):
                                try:
                                    p = next(next_gen)
                                except StopIteration:
                                    next_gen = None
                                    break
                                r = run_piece(p)
                                gx_next = r if r is not None else gx_next
                flush_pending()
                tgt_piece(5)
                nc.sync.dma_start(out_l[:], l_sb[:])

            # ============ fp8 DoubleRow vocab logits + sum-exp ============
            with tc.tile_pool(name="psL", bufs=2,
                              space=bass.MemorySpace.PSUM) as psL:
                s_all = pconst.tile([128, SBC], f32)

                for sb in range(SBC):
                    st = pfp.tile([128, 512], fp8, tag="st")
                    nc.vector.tensor_scalar_mul(
                        st[:].rearrange("p (k s) -> p k s", k=KC),
                        ht_r[:, :, sb * 128:(sb + 1) * 128], H_SC)
                    sh = []
                    for half in range(2):
                        pl = psL.tile([128, 2048], f32, tag="psL")
                        pe_bias = True
                        if pe_bias:
                            for v in range(4):
                                col = half * 2048 + v * 512
                                nc.tensor.matmul(
                                    pl[:, v * 512:(v + 1) * 512],
                                    ones_row[:], bout[0:1, col:col + 512],
                                    start=True, stop=False)
                        for pair in range(2):
                            lhs = st[:, pair * 256:(pair + 1) * 256]\
                                .rearrange("p (two m) -> p two m", two=2)
                            for v in range(4):
                                col = half * 2048 + v * 512
                                rhs = wof_tiles[pair][:]\
                                    .rearrange("p (two v) -> p two v", two=2)\
                                    [:, :, col:col + 512]
                                nc.tensor.matmul(
                                    pl[:, v * 512:(v + 1) * 512], lhs, rhs,
                                    start=(not pe_bias and pair == 0),
                                    stop=(pair == 1), perf_mode=DR)
                        if not pe_bias:
                            nc.vector.tensor_add(
                                pl[:], pl[:],
                                bout[:, half * 2048:half * 2048 + 2048])
                        sh_t = plog.tile([128, 1], f32, tag=f"sh{half}",
                                         name="sh_t")
                        nc.scalar.activation(pl[:], pl[:], AF.Exp,
                                             scale=1.0 / L_SC,
                                             accum_out=sh_t[:])
                        sh.append(sh_t)
                    nc.gpsimd.tensor_add(s_all[:, sb:sb + 1],
                                         sh[0][:], sh[1][:])
                nc.sync.dma_start(out_s[:], s_all[:])

    nc.compile()
    return nc


def _prep(inputs):
    """Host-side data prep. Returns per-core in_maps + host combine data."""
    il = np.asarray(inputs["input_lines"])
    tl = np.asarray(inputs["target_lines"])
    f = lambda k: np.asarray(inputs[k], np.float32)
    emb_in, emb_tgt = f("emb_in").copy(), f("emb_tgt").copy()
    emb_in[0] = 0.0
    emb_tgt[0] = 0.0
    W_out, b_out = f("W_out"), f("b_out")

    perm = np.concatenate([np.arange(0, 512), np.arange(512, 1024),
                           np.arange(1536, 2048), np.arange(1024, 1536)])

    def wt(w):  # [2048,512] -> [4,128,2048] bf16 (transposed, gate-permuted)
        return np.ascontiguousarray(
            w[perm].T.reshape(KC, 128, 4 * H)).astype(BF16)

    def wt_pair(w):  # -> [2,128,2*2048] fp8 (k-pairs side by side, scaled)
        r = np.clip(w[perm].T * WI_SC, -240, 240).reshape(KC, 128, 4 * H)
        pairs = np.stack([
            np.concatenate([r[0], r[1]], axis=1),
            np.concatenate([r[2], r[3]], axis=1)])
        return np.ascontiguousarray(pairs).astype(FP8)

    def bias(bi, bh):  # -> [128, 16] f32
        return np.ascontiguousarray(
            (bi + bh)[perm].reshape(NG, 128).T).astype(np.float32)

    x_enc = emb_in[il.reshape(-1)]                       # [3072, 512]
    xt_enc = np.ascontiguousarray(x_enc.T).astype(BF16)  # [512, 3072]
    tgt_in = tl[:DEC].reshape(-1)
    x_dec = emb_tgt[tgt_in]
    xt_dec = np.ascontiguousarray(x_dec.T).astype(BF16)  # [512, 3008]

    m = (il == 0).astype(np.uint8)                       # [48, 64]
    mask = np.ascontiguousarray(np.broadcast_to(
        m[:, None, None, :], (SRC, 128, KC, B)).reshape(
            SRC, 128, KC * B)).astype(np.uint8)

    tgt_next = tl[1:TGT].reshape(-1)                     # [3008]
    wtgt = np.ascontiguousarray(np.clip(
        W_out[tgt_next].T * W_SC, -240, 240).reshape(
            KC, 128, SB)).astype(FP8)
    b_tgt = b_out[tgt_next].astype(np.float64)

    common = dict(
        xt_enc=xt_enc, xt_dec=xt_dec,
        wi_e=wt(f("W_ih_e")), wh_e=wt(f("W_hh_e")),
        wi_d=wt(f("W_ih_d")), wh_d=wt(f("W_hh_d")),
        bias_e=bias(f("b_ih_e"), f("b_hh_e")),
        bias_d=bias(f("b_ih_d"), f("b_hh_d")),
        mask=mask, wtgt=wtgt,
        ident=np.eye(128, dtype=BF16),
    )
    in_maps = []
    for c in range(NCORES):
        ws = np.zeros((VSP, H), np.float32)
        ws[:VSH] = W_out[c * VSH:(c + 1) * VSH]
        bs = np.full(VSP, -88.0 * L_SC, np.float32)
        bs[:VSH] = b_out[c * VSH:(c + 1) * VSH] * L_SC
        in_maps.append(dict(
            common,
            wot=np.ascontiguousarray(np.clip(
                ws.T * W_SC, -240, 240).reshape(KC, 128, VSP)).astype(FP8),
            bout_sc=np.ascontiguousarray(
                np.broadcast_to(bs, (128, VSP))).astype(BF16),
        ))
    return in_maps, b_tgt


def _combine(results, b_tgt):
    s = np.zeros(SBP, np.float64)
    for r in results:
        s += np.asarray(r["out_s"], np.float64).T.reshape(-1)
    s = s[:SB]
    lse = np.log(s)
    l_tgt = np.asarray(results[0]["out_l"], np.float64).reshape(-1) + b_tgt
    return np.float32((lse - l_tgt).sum() / B)


def kernel(**inputs):
    global _COMPILED
    from concourse.bass_utils import run_bass_kernel_spmd
    in_maps, b_tgt = _prep(inputs)
    if _COMPILED is None:
        _COMPILED = _build()
    res = run_bass_kernel_spmd(_COMPILED, in_maps, list(range(NCORES)))
    return _combine(res.results, b_tgt)


if __name__ == "__main__":
    import reference
    inp = reference.setup_inputs()
    expected = np.asarray(reference.reference(**inp))
    actual = kernel(**{k: np.asarray(v) for k, v in inp.items()})
    err = abs(actual - expected) / max(abs(expected), 1e-9)
    print(f"expected={expected} actual={actual} rel_err={err:.3e}")
